# revision 32
# baseline (speedup 1.0000x reference)
"""Trainium2 Bass kernel for nn_DeepQNetwork (conv encoder + 8-expert MLP head).

Strategy: data-parallel over 8 NeuronCores (256 samples each). Convs are
mapped to TensorE matmuls via host-side space-to-depth (stride-s conv ->
s*s-folded channels, kernel split into 2x2 accumulation steps). The expert
MLP computes all 8 experts (expert-paired on the M dim, block-diagonal
weights for the 64x64 layers); the per-sample expert row is gathered on the
host. bf16 operands with fp32 PSUM accumulation.

Self-contained: only concourse/numpy imports, shapes hardcoded.
"""

import os
import sys

import ml_dtypes
import numpy as np

BF16 = ml_dtypes.bfloat16

B, E, A = 2048, 8, 6
NCORES = 8
BC = B // NCORES  # 256 samples per core
CHUNK = 32  # conv1/conv2 batch chunk (16 per half)
HALF = 16
# chunks whose conv1 input ships without the kx-fold duplication (half the
# DMA bytes, 4 instead of 2 PSUM-accumulation steps)
NODUP = (0, 1, 2)


def _install_axon_prof_shim():
    """Register the NTFF profile hook (exec-time measurement) under axon."""
    import sys
    import types

    if "antenv.axon_hooks" not in sys.modules:
        mod = types.ModuleType("antenv.axon_hooks")
        _hook = [None]
        mod.set_axon_ntff_profile_hook = lambda h: _hook.__setitem__(0, h)
        mod.get_axon_ntff_profile_hook = lambda: _hook[0]
        sys.modules["antenv.axon_hooks"] = mod
        import antenv

        antenv.axon_hooks = mod
    from antenv.axon_hooks import (
        get_axon_ntff_profile_hook,
        set_axon_ntff_profile_hook,
    )

    if get_axon_ntff_profile_hook() is None:
        try:
            from trn_agent_boot.trn_boot import _ntff_profile_via_ctypes

            set_axon_ntff_profile_hook(
                _ntff_profile_via_ctypes("/opt/axon/libaxon_pjrt.so")
            )
        except Exception:
            pass
    import concourse.bass_utils as bu

    bu.upload_artifacts = lambda tmpdir: tmpdir


def _build_program(wins, nodup=()):
    import concourse.mybir as mybir
    import concourse.tile as tile
    from concourse import bacc

    dt = mybir.dt
    AF = mybir.ActivationFunctionType
    ALU = mybir.AluOpType

    WMAX = max(e0 - s0 for s0, e0 in wins)
    nodup = frozenset(nodup)

    nc = bacc.Bacc(
        "TRN2", target_bir_lowering=False, debug=False, num_devices=NCORES
    )

    # ---- DRAM tensors ----
    x1a_d = nc.dram_tensor("x1a", [96, 128, 21, 21], dt.bfloat16, kind="ExternalInput")
    x1b_d = nc.dram_tensor("x1b", [96, 128, 21, 21], dt.bfloat16, kind="ExternalInput")
    x1an_d = nc.dram_tensor("x1an", [48, 128, 21, 21], dt.bfloat16, kind="ExternalInput")
    x1bn_d = nc.dram_tensor("x1bn", [48, 128, 21, 21], dt.bfloat16, kind="ExternalInput")
    k1_d = nc.dram_tensor("k1r", [96, 64], dt.bfloat16, kind="ExternalInput")
    k1n_d = nc.dram_tensor("k1n", [48, 128], dt.bfloat16, kind="ExternalInput")
    k2_d = nc.dram_tensor("k2r", [128, 256], dt.bfloat16, kind="ExternalInput")
    k3_d = nc.dram_tensor("k3r", [128, 384], dt.bfloat16, kind="ExternalInput")
    w1_d = nc.dram_tensor("w1r", [128, 4 * 25 * 128], dt.bfloat16, kind="ExternalInput")
    w25_d = nc.dram_tensor("w25r", [128, 16 * 128], dt.bfloat16, kind="ExternalInput")
    w6_d = nc.dram_tensor("w6r", [128, 48], dt.bfloat16, kind="ExternalInput")
    c1_d = nc.dram_tensor("c1t", [128, 1], dt.float32, kind="ExternalInput")
    c2_d = nc.dram_tensor("c2t", [64, 1], dt.float32, kind="ExternalInput")
    c3_d = nc.dram_tensor("c3t", [64, 1], dt.float32, kind="ExternalInput")
    bm_d = nc.dram_tensor("bmlp", [128, 20], dt.float32, kind="ExternalInput")
    b6_d = nc.dram_tensor("b6t", [12, 4], dt.float32, kind="ExternalInput")
    out_d = nc.dram_tensor("out", [12, 4, WMAX], dt.float32, kind="ExternalOutput")
    debug = bool(os.environ.get("NN_KERNEL_DEBUG"))
    if debug:
        dbgx2_d = nc.dram_tensor("dbg_x2", [128, 32, 10, 10], dt.bfloat16, kind="ExternalOutput")
        dbgx3_d = nc.dram_tensor("dbg_x3", [128, 256, 9, 9], dt.bfloat16, kind="ExternalOutput")
        dbgx4_d = nc.dram_tensor("dbg_x4", [128, 25, 256], dt.bfloat16, kind="ExternalOutput")
        dbgh1_d = nc.dram_tensor("dbg_h1", [128, 4, 256], dt.bfloat16, kind="ExternalOutput")
        dbgh5_d = nc.dram_tensor("dbg_h5", [128, 4, 256], dt.bfloat16, kind="ExternalOutput")

    with tile.TileContext(nc) as tc:
        with (
            tc.tile_pool(name="wts", bufs=1) as wpool,
            tc.tile_pool(name="x1", bufs=3) as x1pool,
            tc.tile_pool(name="x2", bufs=2) as x2pool,
            tc.tile_pool(name="big", bufs=1) as bigpool,
            tc.tile_pool(name="psc", bufs=6, space="PSUM") as psc,
            tc.tile_pool(name="psm", bufs=2, space="PSUM") as psm,
        ):
            # ---- chunk-0 first input slices, then conv weights ----
            # 4-sample slices (a/b interleaved to match conv1 g8 order) let
            # conv1 begin on partial arrival.
            dedup0 = 0 in nodup
            rows0 = 48 if dedup0 else 96
            xa0_d, xb0_d = (x1an_d, x1bn_d) if dedup0 else (x1a_d, x1b_d)
            X1ta0 = x1pool.tile([rows0, HALF, 21, 21], dt.bfloat16, tag="x1a")
            X1tb0 = x1pool.tile([rows0, HALF, 21, 21], dt.bfloat16, tag="x1b")
            nc.sync.dma_start(X1ta0[:, 0:4], xa0_d.ap()[:, 0:4])
            k1sb = wpool.tile([96, 64], dt.bfloat16)
            nc.sync.dma_start(k1sb[:], k1_d.ap())
            k1nsb = wpool.tile([48, 128], dt.bfloat16)
            if nodup:
                nc.sync.dma_start(k1nsb[:], k1n_d.ap())
            nc.sync.dma_start(X1tb0[:, 0:4], xb0_d.ap()[:, 0:4])
            c1sb = wpool.tile([128, 1], dt.float32)
            nc.sync.dma_start(c1sb[:], c1_d.ap())
            for s in range(1, 4):
                nc.sync.dma_start(
                    X1ta0[:, 4 * s : 4 * s + 4], xa0_d.ap()[:, 4 * s : 4 * s + 4]
                )
                nc.sync.dma_start(
                    X1tb0[:, 4 * s : 4 * s + 4], xb0_d.ap()[:, 4 * s : 4 * s + 4]
                )

            k2sb = wpool.tile([128, 256], dt.bfloat16)
            nc.sync.dma_start(k2sb[:], k2_d.ap())
            k3sb = wpool.tile([128, 384], dt.bfloat16)
            nc.sync.dma_start(k3sb[:], k3_d.ap())
            c2sb = wpool.tile([64, 1], dt.float32)
            nc.sync.dma_start(c2sb[:], c2_d.ap())
            c3sb = wpool.tile([64, 1], dt.float32)
            nc.sync.dma_start(c3sb[:], c3_d.ap())

            # MLP weight tiles (DMAs interleaved into the chunk loop below so
            # w1 lands before the scheduler-hoisted MLP1 windows need it,
            # without front-running the chunk-0/1 input loads)
            w1sb = wpool.tile([128, 4 * 25 * 128], dt.bfloat16)
            w25sb = wpool.tile([128, 16 * 128], dt.bfloat16)
            w6sb = wpool.tile([128, 48], dt.bfloat16)
            bmsb = wpool.tile([128, 20], dt.float32)
            b6sb = wpool.tile([12, 4], dt.float32)

            def emit_wdma(c):
                if 1 <= c <= 4:
                    p = c - 1
                    nc.sync.dma_start(
                        w1sb[:, p * 3200 : (p + 1) * 3200],
                        w1_d.ap()[:, p * 3200 : (p + 1) * 3200],
                    )
                elif c == 5:
                    nc.sync.dma_start(w25sb[:], w25_d.ap())
                    nc.sync.dma_start(w6sb[:], w6_d.ap())
                    nc.sync.dma_start(bmsb[:], bm_d.ap())
                    nc.sync.dma_start(b6sb[:], b6_d.ap())

            X3t = bigpool.tile([128, 256, 9, 9], dt.bfloat16)
            X4t = bigpool.tile([128, 25, 256], dt.bfloat16)
            Ha = bigpool.tile([128, 4, WMAX], dt.bfloat16)
            Hb = bigpool.tile([128, 4, WMAX], dt.bfloat16)
            OUTt = bigpool.tile([12, 4, WMAX], dt.float32)

            copy_ctr = 0

            def relu_copy(dst, src, bias):
                nonlocal copy_ctr
                copy_ctr += 1
                r = copy_ctr % 3
                if r == 0:
                    nc.scalar.activation(dst, src, AF.Relu, bias=bias)
                elif r == 1:
                    nc.vector.tensor_scalar(dst, src, bias, 0.0, ALU.add, ALU.max)
                else:
                    nc.gpsimd.tensor_scalar(dst, src, bias, 0.0, ALU.add, ALU.max)

            nc.vector.memset(X4t[64:128, 24:25, :], 0.0)

            # ---- conv3 per-group body (interleaved into the chunk loop) ----
            def emit_conv3(gp):
                pt3p = psc.tile([128, 8, 7, 7], dt.float32, tag="conv")
                for kx in range(3):
                    for q in range(2):
                        g = 2 * gp + q
                        nc.tensor.matmul(
                            pt3p[64 * q : 64 * q + 64],
                            k3sb[:, 64 * kx : 64 * kx + 64],
                            X3t[:, 8 * g : 8 * g + 8, 0:7, kx : kx + 7],
                            start=(kx == 0),
                            stop=False,
                            tile_position=(0, 64 * q),
                        )
                    for q in range(2):
                        g = 2 * gp + q
                        nc.tensor.matmul(
                            pt3p[64 * q : 64 * q + 64],
                            k3sb[0:64, 192 + 64 * kx : 192 + 64 * kx + 64],
                            X3t[0:64, 8 * g : 8 * g + 8, 2:9, kx : kx + 7],
                            start=False,
                            stop=(kx == 2),
                            tile_position=(0, 64 * q),
                        )
                for q in range(2):
                    g = 2 * gp + q
                    ptf = pt3p[64 * q : 64 * q + 64].rearrange("p b y x -> p b (y x)")
                    relu_copy(
                        X4t[0:64, 0:25, 8 * g : 8 * g + 8],
                        ptf[:, :, 0:49:2].rearrange("p b s -> p s b"),
                        c3sb[:],
                    )
                    relu_copy(
                        X4t[64:128, 0:24, 8 * g : 8 * g + 8],
                        ptf[:, :, 1:49:2].rearrange("p b s -> p s b"),
                        c3sb[:],
                    )

            # ---- conv1 + conv2 + conv3, chunked over batch ----
            for c in range(8):
                dedup = c in nodup
                if c == 0:
                    X1ta, X1tb = X1ta0, X1tb0
                else:
                    rows = 48 if dedup else 96
                    xsrc = (
                        ((x1an_d, x1bn_d)) if dedup else ((x1a_d, x1b_d))
                    )
                    X1ta = x1pool.tile([rows, HALF, 21, 21], dt.bfloat16, tag="x1a")
                    X1tb = x1pool.tile([rows, HALF, 21, 21], dt.bfloat16, tag="x1b")
                    for X1t, xd in ((X1ta, xsrc[0]), (X1tb, xsrc[1])):
                        for s in range(4):
                            nc.sync.dma_start(
                                X1t[:, 4 * s : 4 * s + 4],
                                xd.ap()[
                                    :, c * HALF + 4 * s : c * HALF + 4 * s + 4
                                ],
                            )
                emit_wdma(c)
                X2t = x2pool.tile([128, CHUNK, 10, 10], dt.bfloat16)

                # conv1: dup chunks fold kx into K=96 (2 ky steps); dedup
                # chunks use the 48-row base layout with 4 (ky,kx) steps
                for g8 in range(8):
                    h, g = g8 % 2, g8 // 2
                    X1h = X1ta if h == 0 else X1tb
                    pt = psc.tile([128, 4, 10, 10], dt.float32, tag="conv")
                    nsteps = 4 if dedup else 2
                    for a in range(nsteps):
                        for q in range(4):
                            di, dj = q >> 1, q & 1
                            if dedup:
                                ky, kx = a >> 1, a & 1
                                rhs = X1h[
                                    :,
                                    4 * g : 4 * g + 4,
                                    (di + ky) : (di + ky + 19) : 2,
                                    (dj + kx) : (dj + kx + 19) : 2,
                                ]
                                lhs = k1nsb[:, 32 * a : 32 * a + 32]
                            else:
                                rhs = X1h[
                                    :,
                                    4 * g : 4 * g + 4,
                                    (di + a) : (di + a + 19) : 2,
                                    dj : dj + 19 : 2,
                                ]
                                lhs = k1sb[:, 32 * a : 32 * a + 32]
                            nc.tensor.matmul(
                                pt[32 * q : 32 * q + 32],
                                lhs,
                                rhs,
                                start=(a == 0),
                                stop=(a == nsteps - 1),
                                tile_position=(0, 32 * q),
                            )
                    relu_copy(X2t[:, HALF * h + 4 * g : HALF * h + 4 * g + 4], pt[:], c1sb[:])

                if debug and c == 0:
                    nc.sync.dma_start(dbgx2_d.ap(), X2t[:])

                # conv2: K=128 x 4 steps, two 4-sample groups on col-tiles
                for gp in range(4):
                    ptp = psc.tile([128, 4, 9, 9], dt.float32, tag="conv")
                    for s in range(4):
                        a, a2 = s >> 1, s & 1
                        for q in range(2):
                            g = 2 * gp + q
                            rhs = X2t[:, 4 * g : 4 * g + 4, a : a + 9, a2 : a2 + 9]
                            nc.tensor.matmul(
                                ptp[64 * q : 64 * q + 64],
                                k2sb[:, 64 * s : 64 * s + 64],
                                rhs,
                                start=(s == 0),
                                stop=(s == 3),
                                tile_position=(0, 64 * q),
                            )
                    for q in range(2):
                        b0 = c * CHUNK + 4 * (2 * gp + q)
                        pq = ptp[64 * q : 64 * q + 64]
                        relu_copy(X3t[0:64, b0 : b0 + 4], pq, c2sb[:])
                        relu_copy(
                            X3t[64:128, b0 : b0 + 4, 0:8, :],
                            pq[:, :, 1:9, :],
                            c2sb[:],
                        )

                # conv3 for this chunk's samples fills tensor-engine time
                # while the next chunk's input DMA completes
                emit_conv3(2 * c)
                emit_conv3(2 * c + 1)

            if debug:
                nc.sync.dma_start(dbgx3_d.ap(), X3t[:])
                nc.sync.dma_start(dbgx4_d.ap(), X4t[:])

            # ---- MLP layer 1: expert-pair windows over the sorted batch ----
            # Samples arrive expert-sorted; pair p's window covers experts
            # 2p,2p+1 on every core (block-diagonal weights pick the correct
            # expert rows; off-pair overlap columns produce ignored garbage).
            for p in range(4):
                s0, e0 = wins[p]
                Wp = e0 - s0
                if Wp <= 0:
                    continue
                ptm = psm.tile([128, Wp], dt.float32, tag="mlp")
                for t in range(25):
                    nc.tensor.matmul(
                        ptm[:],
                        w1sb[:, (p * 25 + t) * 128 : (p * 25 + t) * 128 + 128],
                        X4t[:, t, s0:e0],
                        start=(t == 0),
                        stop=(t == 24),
                    )
                relu_copy(Ha[:, p, 0:Wp], ptm[:], bmsb[:, p : p + 1])

            # ---- MLP layers 2-5: block-diagonal expert pairs ----
            src, dst = Ha, Hb
            for l in range(4):
                for p in range(4):
                    s0, e0 = wins[p]
                    Wp = e0 - s0
                    if Wp <= 0:
                        continue
                    idx = l * 4 + p
                    ptm = psm.tile([128, Wp], dt.float32, tag="mlp")
                    nc.tensor.matmul(
                        ptm[:],
                        w25sb[:, 128 * idx : 128 * idx + 128],
                        src[:, p, 0:Wp],
                        start=True,
                        stop=True,
                    )
                    relu_copy(dst[:, p, 0:Wp], ptm[:], bmsb[:, (l + 1) * 4 + p : (l + 1) * 4 + p + 1])
                src, dst = dst, src

            # ---- MLP layer 6 (no relu) + incremental output DMA ----
            for p in range(4):
                s0, e0 = wins[p]
                Wp = e0 - s0
                if Wp <= 0:
                    continue
                pt6 = psm.tile([12, Wp], dt.float32, tag="mlp")
                nc.tensor.matmul(
                    pt6[:],
                    w6sb[:, 12 * p : 12 * p + 12],
                    src[:, p, 0:Wp],
                    start=True,
                    stop=True,
                )
                nc.vector.tensor_scalar(
                    OUTt[:, p, 0:Wp], pt6[:], b6sb[:, p : p + 1], None, ALU.add
                )
                nc.sync.dma_start(out_d.ap()[:, p, 0:Wp], OUTt[:, p, 0:Wp])

    nc.compile()
    return nc


_prog_cache = {}
LAST_RESULTS = None


def _expert_layout(rm_state):
    """Balanced expert->core assignment + per-core expert-sorted sample order.

    Returns (perm, core_samples, ocum, wins):
      perm[2048]      sample permutation; perm[k*256:(k+1)*256] = core k's
                      samples, sorted by expert
      core_samples    [core][expert] -> global sample indices (in order)
      ocum[E+1,cores] per-core cumulative group offsets
      wins            4 (start,end) compile-time column windows; on every
                      core, experts 2p,2p+1 lie within wins[p]
    """
    n = np.bincount(rm_state, minlength=E).astype(np.int64)
    base = n // NCORES
    extra = np.zeros((E, NCORES), np.int64)
    ptr = 0
    for e in range(E):
        for _ in range(int(n[e] % NCORES)):
            extra[e, ptr % NCORES] += 1
            ptr += 1
    Mct = base[:, None] + extra  # [E, NCORES]; column sums == 256
    order = np.argsort(rm_state, kind="stable")
    core_samples = [[] for _ in range(NCORES)]
    off = 0
    for e in range(E):
        idxs = order[off : off + n[e]]
        off += int(n[e])
        s = 0
        for k in range(NCORES):
            core_samples[k].append(idxs[s : s + Mct[e, k]])
            s += int(Mct[e, k])
    perm = np.concatenate([np.concatenate(cs) for cs in core_samples])
    ocum = np.zeros((E + 1, NCORES), np.int64)
    ocum[1:] = np.cumsum(Mct, axis=0)
    Bcum = np.zeros(E + 1, np.int64)
    Bcum[1:] = np.cumsum(base)
    wins = []
    for p in range(4):
        s0 = int(Bcum[2 * p])
        e0 = int(min(Bcum[2 * p + 2] + 2 * p + 2, BC))
        wins.append((s0, max(s0, e0)))
    return perm, core_samples, ocum, wins


def kernel(**inputs):
    state = np.asarray(inputs["state"], dtype=np.float32)
    rm_state = np.asarray(inputs["rm_state"]).astype(np.int64)
    perm, core_samples, ocum, wins = _expert_layout(rm_state)
    state = state[perm]
    k1 = np.asarray(inputs["k1"], dtype=np.float32)
    c1 = np.asarray(inputs["c1"], dtype=np.float32)
    k2 = np.asarray(inputs["k2"], dtype=np.float32)
    c2 = np.asarray(inputs["c2"], dtype=np.float32)
    k3 = np.asarray(inputs["k3"], dtype=np.float32)
    c3 = np.asarray(inputs["c3"], dtype=np.float32)
    Ws = [np.asarray(inputs[f"W{i}"], dtype=np.float32) for i in range(1, 7)]
    Bs = [np.asarray(inputs[f"B{i}"], dtype=np.float32) for i in range(1, 7)]

    # ---- host prep ----
    # space-to-depth: [core, b, (i,dy), (j,dx), c] -> [core, (dy,dx,c), b, i, j]
    v = state.reshape(NCORES, BC, 21, 4, 21, 4, 3)
    s2d = np.ascontiguousarray(np.transpose(v, (0, 3, 5, 6, 1, 2, 4))).reshape(
        NCORES, 48, BC, 21, 21
    )
    # chunk c processes samples [32c, 32c+32): first 16 via x1a, next 16 via
    # x1b. Rows 48-95 duplicate rows 0-47 shifted one column in j (folds the
    # kx phase into the contraction dim -> K=96, 2 accumulation steps).
    vv = s2d.reshape(NCORES, 48, 8, 2, 16, 21, 21)
    halves = []
    bases = []
    for hh in range(2):
        base = np.ascontiguousarray(vv[:, :, :, hh]).reshape(NCORES, 48, 128, 21, 21)
        bases.append(base.astype(BF16))
        dup = np.zeros((NCORES, 96, 128, 21, 21), np.float32)
        dup[:, 0:48] = base
        dup[:, 48:96, :, :, 0:20] = base[:, :, :, :, 1:21]
        halves.append(dup.astype(BF16))
    x1a, x1b = halves
    x1an, x1bn = bases

    # conv weights: partition=(dy,dx,ci), free=(step, co)
    k1s = (
        k1.reshape(32, 3, 2, 4, 2, 4)
        .transpose(2, 4, 3, 5, 1, 0)
        .reshape(4, 48, 32)
    )  # steps (a, a2)
    k1r = np.zeros((96, 64), np.float32)
    for a in range(2):
        k1r[0:48, 32 * a : 32 * a + 32] = k1s[2 * a]
        k1r[48:96, 32 * a : 32 * a + 32] = k1s[2 * a + 1]
    k1r = k1r.astype(BF16)
    # base-layout conv1 weights: 4 (ky,kx) steps, K=48
    k1n = np.ascontiguousarray(k1s.transpose(1, 0, 2)).reshape(48, 128).astype(BF16)
    k2r = (
        k2.reshape(64, 32, 2, 2, 2, 2)
        .transpose(2, 4, 3, 5, 1, 0)
        .reshape(4, 128, 64)
        .transpose(1, 0, 2)
        .reshape(128, 256)
        .astype(BF16)
    )
    t3 = k3.transpose(2, 3, 1, 0)  # [ky, kx, ci, co]
    k3r = np.zeros((128, 384), np.float32)
    for kx in range(3):
        k3r[0:64, 64 * kx : 64 * kx + 64] = t3[0, kx]
        k3r[64:128, 64 * kx : 64 * kx + 64] = t3[1, kx]
        k3r[0:64, 192 + 64 * kx : 192 + 64 * kx + 64] = t3[2, kx]
    k3r = k3r.astype(BF16)

    # W1: [E, 3136, 64] -> s-pair tiles [partition=(spar,c), (pair, t, e2*64+m)]
    w1v = np.ascontiguousarray(
        Ws[0].reshape(4, 2, 64, 49, 64).transpose(0, 3, 2, 1, 4)
    ).reshape(4, 49, 64, 128)
    w1r = np.zeros((4, 25, 128, 128), np.float32)
    w1r[:, :, 0:64] = w1v[:, 0::2]
    w1r[:, :24, 64:128] = w1v[:, 1::2]
    # -> [128, 4*25*128] partition-major
    w1r = np.ascontiguousarray(w1r.transpose(2, 0, 1, 3)).reshape(128, 4 * 25 * 128).astype(BF16)

    # W2..W5 block-diagonal expert pairs: [128, 16*128] (col = (l*4+p)*128 + m')
    w25 = np.zeros((128, 16, 128), np.float32)
    for l in range(4):
        Wl = Ws[1 + l]
        for p in range(4):
            for e2 in range(2):
                w25[64 * e2 : 64 * e2 + 64, l * 4 + p, 64 * e2 : 64 * e2 + 64] = Wl[
                    2 * p + e2
                ]
    w25 = w25.reshape(128, 16 * 128).astype(BF16)

    w6 = np.zeros((128, 4, 12), np.float32)
    for p in range(4):
        for e2 in range(2):
            w6[64 * e2 : 64 * e2 + 64, p, 6 * e2 : 6 * e2 + 6] = Ws[5][2 * p + e2]
    w6 = w6.reshape(128, 48).astype(BF16)

    c1t = np.tile(c1, 4)[:, None].astype(np.float32)
    c2t = c2[:, None].astype(np.float32)
    c3t = c3[:, None].astype(np.float32)
    bmlp = np.zeros((128, 20), np.float32)
    for l in range(5):
        Bl = Bs[l]
        for p in range(4):
            bmlp[:, l * 4 + p] = np.concatenate([Bl[2 * p], Bl[2 * p + 1]])
    b6t = np.zeros((12, 4), np.float32)
    for p in range(4):
        b6t[:, p] = np.concatenate([Bs[5][2 * p], Bs[5][2 * p + 1]])

    # ---- build + run ----
    trace = bool(os.environ.get("NN_KERNEL_TRACE"))
    _install_axon_prof_shim()
    key = (tuple(wins), NODUP)
    if key not in _prog_cache:
        _prog_cache[key] = _build_program(wins, nodup=NODUP)
    nc = _prog_cache[key]

    shared = {
        "k1r": k1r,
        "k1n": k1n,
        "k2r": k2r,
        "k3r": k3r,
        "w1r": w1r,
        "w25r": w25,
        "w6r": w6,
        "c1t": c1t,
        "c2t": c2t,
        "c3t": c3t,
        "bmlp": bmlp,
        "b6t": b6t,
    }
    in_maps = [
        {
            "x1a": np.ascontiguousarray(x1a[c]),
            "x1b": np.ascontiguousarray(x1b[c]),
            "x1an": np.ascontiguousarray(x1an[c]),
            "x1bn": np.ascontiguousarray(x1bn[c]),
            **shared,
        }
        for c in range(NCORES)
    ]

    from concourse.bass_utils import run_bass_kernel_spmd

    res = run_bass_kernel_spmd(
        nc, in_maps, core_ids=list(range(NCORES)), trace=trace
    )
    if trace and res.exec_time_ns is not None:
        print(f"HW exec time: {res.exec_time_ns} ns")

    global LAST_RESULTS
    LAST_RESULTS = res.results

    # ---- host gather: pick each sample's own expert rows/cols ----
    WMAX = max(e0 - s0 for s0, e0 in wins)
    result = np.zeros((B, A), np.float32)
    for k in range(NCORES):
        out_k = np.asarray(res.results[k]["out"]).reshape(12, 4, WMAX)
        for e in range(E):
            idxs = core_samples[k][e]
            if len(idxs) == 0:
                continue
            p = e // 2
            s0, _ = wins[p]
            col0 = int(ocum[e, k]) - s0
            rows = slice((e % 2) * A, (e % 2) * A + A)
            result[idxs] = out_k[rows, p, col0 : col0 + len(idxs)].T
    return result



# revision 35
# speedup vs baseline: 1.0441x; 1.0441x over previous
"""Trainium2 Bass kernel for nn_DeepQNetwork (conv encoder + 8-expert MLP head).

Strategy: data-parallel over 8 NeuronCores (256 samples each). Convs are
mapped to TensorE matmuls via host-side space-to-depth (stride-s conv ->
s*s-folded channels, kernel split into 2x2 accumulation steps). The expert
MLP computes all 8 experts (expert-paired on the M dim, block-diagonal
weights for the 64x64 layers); the per-sample expert row is gathered on the
host. bf16 operands with fp32 PSUM accumulation.

Self-contained: only concourse/numpy imports, shapes hardcoded.
"""

import os
import sys

import ml_dtypes
import numpy as np

BF16 = ml_dtypes.bfloat16

B, E, A = 2048, 8, 6
NCORES = 8
BC = B // NCORES  # 256 samples per core
CHUNK = 32  # conv1/conv2 batch chunk (16 per half)
HALF = 16
# chunks whose conv1 input ships without the kx-fold duplication (half the
# DMA bytes, 4 instead of 2 PSUM-accumulation steps)
NODUP = (0, 1)


def _install_axon_prof_shim():
    """Register the NTFF profile hook (exec-time measurement) under axon."""
    import sys
    import types

    if "antenv.axon_hooks" not in sys.modules:
        mod = types.ModuleType("antenv.axon_hooks")
        _hook = [None]
        mod.set_axon_ntff_profile_hook = lambda h: _hook.__setitem__(0, h)
        mod.get_axon_ntff_profile_hook = lambda: _hook[0]
        sys.modules["antenv.axon_hooks"] = mod
        import antenv

        antenv.axon_hooks = mod
    from antenv.axon_hooks import (
        get_axon_ntff_profile_hook,
        set_axon_ntff_profile_hook,
    )

    if get_axon_ntff_profile_hook() is None:
        try:
            from trn_agent_boot.trn_boot import _ntff_profile_via_ctypes

            set_axon_ntff_profile_hook(
                _ntff_profile_via_ctypes("/opt/axon/libaxon_pjrt.so")
            )
        except Exception:
            pass
    import concourse.bass_utils as bu

    bu.upload_artifacts = lambda tmpdir: tmpdir


def _build_program(wins, nodup=()):
    import concourse.mybir as mybir
    import concourse.tile as tile
    from concourse import bacc

    dt = mybir.dt
    AF = mybir.ActivationFunctionType
    ALU = mybir.AluOpType

    WMAX = max(e0 - s0 for s0, e0 in wins)
    nodup = frozenset(nodup)

    nc = bacc.Bacc(
        "TRN2", target_bir_lowering=False, debug=False, num_devices=NCORES
    )

    # ---- DRAM tensors ----
    x1a_d = nc.dram_tensor("x1a", [96, 128, 21, 21], dt.bfloat16, kind="ExternalInput")
    x1b_d = nc.dram_tensor("x1b", [96, 128, 21, 21], dt.bfloat16, kind="ExternalInput")
    x1an_d = nc.dram_tensor("x1an", [48, 128, 21, 21], dt.bfloat16, kind="ExternalInput")
    x1bn_d = nc.dram_tensor("x1bn", [48, 128, 21, 21], dt.bfloat16, kind="ExternalInput")
    k1_d = nc.dram_tensor("k1r", [96, 64], dt.bfloat16, kind="ExternalInput")
    k1n_d = nc.dram_tensor("k1n", [48, 128], dt.bfloat16, kind="ExternalInput")
    k2_d = nc.dram_tensor("k2r", [128, 256], dt.bfloat16, kind="ExternalInput")
    k3_d = nc.dram_tensor("k3r", [128, 384], dt.bfloat16, kind="ExternalInput")
    w1_d = nc.dram_tensor("w1r", [128, 4 * 25 * 128], dt.bfloat16, kind="ExternalInput")
    w25_d = nc.dram_tensor("w25r", [128, 16 * 128], dt.bfloat16, kind="ExternalInput")
    w6_d = nc.dram_tensor("w6r", [128, 48], dt.bfloat16, kind="ExternalInput")
    c1_d = nc.dram_tensor("c1t", [128, 1], dt.float32, kind="ExternalInput")
    c2_d = nc.dram_tensor("c2t", [64, 1], dt.float32, kind="ExternalInput")
    c3_d = nc.dram_tensor("c3t", [64, 1], dt.float32, kind="ExternalInput")
    bm_d = nc.dram_tensor("bmlp", [128, 20], dt.float32, kind="ExternalInput")
    b6_d = nc.dram_tensor("b6t", [12, 4], dt.float32, kind="ExternalInput")
    out_d = nc.dram_tensor("out", [12, 4, WMAX], dt.float32, kind="ExternalOutput")
    debug = bool(os.environ.get("NN_KERNEL_DEBUG"))
    if debug:
        dbgx2_d = nc.dram_tensor("dbg_x2", [128, 32, 10, 10], dt.bfloat16, kind="ExternalOutput")
        dbgx3_d = nc.dram_tensor("dbg_x3", [128, 256, 9, 9], dt.bfloat16, kind="ExternalOutput")
        dbgx4_d = nc.dram_tensor("dbg_x4", [128, 25, 256], dt.bfloat16, kind="ExternalOutput")
        dbgh1_d = nc.dram_tensor("dbg_h1", [128, 4, 256], dt.bfloat16, kind="ExternalOutput")
        dbgh5_d = nc.dram_tensor("dbg_h5", [128, 4, 256], dt.bfloat16, kind="ExternalOutput")

    with tile.TileContext(nc) as tc:
        with (
            tc.tile_pool(name="wts", bufs=1) as wpool,
            tc.tile_pool(name="x1", bufs=3) as x1pool,
            tc.tile_pool(name="x2", bufs=2) as x2pool,
            tc.tile_pool(name="big", bufs=1) as bigpool,
            tc.tile_pool(name="psc", bufs=4, space="PSUM") as psc,
            tc.tile_pool(name="psm", bufs=4, space="PSUM") as psm,
        ):
            # ---- chunk-0 first input slices, then conv weights ----
            # 4-sample slices (a/b interleaved to match conv1 g8 order) let
            # conv1 begin on partial arrival.
            dedup0 = 0 in nodup
            rows0 = 48 if dedup0 else 96
            xa0_d, xb0_d = (x1an_d, x1bn_d) if dedup0 else (x1a_d, x1b_d)
            X1ta0 = x1pool.tile([rows0, HALF, 21, 21], dt.bfloat16, tag="x1a")
            X1tb0 = x1pool.tile([rows0, HALF, 21, 21], dt.bfloat16, tag="x1b")
            nc.sync.dma_start(X1ta0[:, 0:4], xa0_d.ap()[:, 0:4])
            k1sb = wpool.tile([96, 64], dt.bfloat16)
            nc.sync.dma_start(k1sb[:], k1_d.ap())
            k1nsb = wpool.tile([48, 128], dt.bfloat16)
            if nodup:
                nc.sync.dma_start(k1nsb[:], k1n_d.ap())
            nc.sync.dma_start(X1tb0[:, 0:4], xb0_d.ap()[:, 0:4])
            c1sb = wpool.tile([128, 1], dt.float32)
            nc.sync.dma_start(c1sb[:], c1_d.ap())
            for s in range(1, 4):
                nc.sync.dma_start(
                    X1ta0[:, 4 * s : 4 * s + 4], xa0_d.ap()[:, 4 * s : 4 * s + 4]
                )
                nc.sync.dma_start(
                    X1tb0[:, 4 * s : 4 * s + 4], xb0_d.ap()[:, 4 * s : 4 * s + 4]
                )

            k2sb = wpool.tile([128, 256], dt.bfloat16)
            nc.sync.dma_start(k2sb[:], k2_d.ap())
            k3sb = wpool.tile([128, 384], dt.bfloat16)
            nc.sync.dma_start(k3sb[:], k3_d.ap())
            c2sb = wpool.tile([64, 1], dt.float32)
            nc.sync.dma_start(c2sb[:], c2_d.ap())
            c3sb = wpool.tile([64, 1], dt.float32)
            nc.sync.dma_start(c3sb[:], c3_d.ap())

            # MLP weight tiles (DMAs interleaved into the chunk loop below so
            # w1 lands before the scheduler-hoisted MLP1 windows need it,
            # without front-running the chunk-0/1 input loads)
            w1sb = wpool.tile([128, 4 * 25 * 128], dt.bfloat16)
            w25sb = wpool.tile([128, 16 * 128], dt.bfloat16)
            w6sb = wpool.tile([128, 48], dt.bfloat16)
            bmsb = wpool.tile([128, 20], dt.float32)
            b6sb = wpool.tile([12, 4], dt.float32)

            def emit_wdma(c):
                if 1 <= c <= 4:
                    p = c - 1
                    nc.sync.dma_start(
                        w1sb[:, p * 3200 : (p + 1) * 3200],
                        w1_d.ap()[:, p * 3200 : (p + 1) * 3200],
                    )
                elif c == 5:
                    nc.sync.dma_start(w25sb[:], w25_d.ap())
                    nc.sync.dma_start(w6sb[:], w6_d.ap())
                    nc.sync.dma_start(bmsb[:], bm_d.ap())
                    nc.sync.dma_start(b6sb[:], b6_d.ap())

            X3t = bigpool.tile([128, 256, 9, 9], dt.bfloat16)
            X4t = bigpool.tile([128, 25, 256], dt.bfloat16)
            Ha = bigpool.tile([128, 4, WMAX], dt.bfloat16)
            Hb = bigpool.tile([128, 4, WMAX], dt.bfloat16)
            OUTt = bigpool.tile([12, 4, WMAX], dt.float32)

            copy_ctr = 0

            def relu_copy(dst, src, bias):
                nonlocal copy_ctr
                copy_ctr += 1
                if copy_ctr % 2 == 0:
                    nc.scalar.activation(dst, src, AF.Relu, bias=bias)
                else:
                    nc.vector.tensor_scalar(dst, src, bias, 0.0, ALU.add, ALU.max)

            nc.vector.memset(X4t[64:128, 24:25, :], 0.0)

            # ---- conv3 per-group body (interleaved into the chunk loop) ----
            def emit_conv3(gp):
                pt3p = psc.tile([128, 8, 7, 7], dt.float32, tag="conv")
                for kx in range(3):
                    for q in range(2):
                        g = 2 * gp + q
                        nc.tensor.matmul(
                            pt3p[64 * q : 64 * q + 64],
                            k3sb[:, 64 * kx : 64 * kx + 64],
                            X3t[:, 8 * g : 8 * g + 8, 0:7, kx : kx + 7],
                            start=(kx == 0),
                            stop=False,
                            tile_position=(0, 64 * q),
                        )
                    for q in range(2):
                        g = 2 * gp + q
                        nc.tensor.matmul(
                            pt3p[64 * q : 64 * q + 64],
                            k3sb[0:64, 192 + 64 * kx : 192 + 64 * kx + 64],
                            X3t[0:64, 8 * g : 8 * g + 8, 2:9, kx : kx + 7],
                            start=False,
                            stop=(kx == 2),
                            tile_position=(0, 64 * q),
                        )
                for q in range(2):
                    g = 2 * gp + q
                    ptf = pt3p[64 * q : 64 * q + 64].rearrange("p b y x -> p b (y x)")
                    relu_copy(
                        X4t[0:64, 0:25, 8 * g : 8 * g + 8],
                        ptf[:, :, 0:49:2].rearrange("p b s -> p s b"),
                        c3sb[:],
                    )
                    relu_copy(
                        X4t[64:128, 0:24, 8 * g : 8 * g + 8],
                        ptf[:, :, 1:49:2].rearrange("p b s -> p s b"),
                        c3sb[:],
                    )

            # ---- conv1 + conv2 + conv3, chunked over batch ----
            for c in range(8):
                dedup = c in nodup
                if c == 0:
                    X1ta, X1tb = X1ta0, X1tb0
                else:
                    rows = 48 if dedup else 96
                    xsrc = (
                        ((x1an_d, x1bn_d)) if dedup else ((x1a_d, x1b_d))
                    )
                    X1ta = x1pool.tile([rows, HALF, 21, 21], dt.bfloat16, tag="x1a")
                    X1tb = x1pool.tile([rows, HALF, 21, 21], dt.bfloat16, tag="x1b")
                    for X1t, xd in ((X1ta, xsrc[0]), (X1tb, xsrc[1])):
                        for s in range(4):
                            nc.sync.dma_start(
                                X1t[:, 4 * s : 4 * s + 4],
                                xd.ap()[
                                    :, c * HALF + 4 * s : c * HALF + 4 * s + 4
                                ],
                            )
                emit_wdma(c)
                X2t = x2pool.tile([128, CHUNK, 10, 10], dt.bfloat16)

                # conv1: dup chunks fold kx into K=96 (2 ky steps); dedup
                # chunks use the 48-row base layout with 4 (ky,kx) steps
                for g8 in range(8):
                    h, g = g8 % 2, g8 // 2
                    X1h = X1ta if h == 0 else X1tb
                    pt = psc.tile([128, 4, 10, 10], dt.float32, tag="conv")
                    nsteps = 4 if dedup else 2
                    for a in range(nsteps):
                        for q in range(4):
                            di, dj = q >> 1, q & 1
                            if dedup:
                                ky, kx = a >> 1, a & 1
                                rhs = X1h[
                                    :,
                                    4 * g : 4 * g + 4,
                                    (di + ky) : (di + ky + 19) : 2,
                                    (dj + kx) : (dj + kx + 19) : 2,
                                ]
                                lhs = k1nsb[:, 32 * a : 32 * a + 32]
                            else:
                                rhs = X1h[
                                    :,
                                    4 * g : 4 * g + 4,
                                    (di + a) : (di + a + 19) : 2,
                                    dj : dj + 19 : 2,
                                ]
                                lhs = k1sb[:, 32 * a : 32 * a + 32]
                            nc.tensor.matmul(
                                pt[32 * q : 32 * q + 32],
                                lhs,
                                rhs,
                                start=(a == 0),
                                stop=(a == nsteps - 1),
                                tile_position=(0, 32 * q),
                            )
                    relu_copy(X2t[:, HALF * h + 4 * g : HALF * h + 4 * g + 4], pt[:], c1sb[:])

                if debug and c == 0:
                    nc.sync.dma_start(dbgx2_d.ap(), X2t[:])

                # conv2: K=128 x 4 steps, two 4-sample groups on col-tiles
                for gp in range(4):
                    ptp = psc.tile([128, 4, 9, 9], dt.float32, tag="conv")
                    for s in range(4):
                        a, a2 = s >> 1, s & 1
                        for q in range(2):
                            g = 2 * gp + q
                            rhs = X2t[:, 4 * g : 4 * g + 4, a : a + 9, a2 : a2 + 9]
                            nc.tensor.matmul(
                                ptp[64 * q : 64 * q + 64],
                                k2sb[:, 64 * s : 64 * s + 64],
                                rhs,
                                start=(s == 0),
                                stop=(s == 3),
                                tile_position=(0, 64 * q),
                            )
                    for q in range(2):
                        b0 = c * CHUNK + 4 * (2 * gp + q)
                        pq = ptp[64 * q : 64 * q + 64]
                        relu_copy(X3t[0:64, b0 : b0 + 4], pq, c2sb[:])
                        relu_copy(
                            X3t[64:128, b0 : b0 + 4, 0:8, :],
                            pq[:, :, 1:9, :],
                            c2sb[:],
                        )

                # conv3 for this chunk's samples fills tensor-engine time
                # while the next chunk's input DMA completes
                emit_conv3(2 * c)
                emit_conv3(2 * c + 1)

            if debug:
                nc.sync.dma_start(dbgx3_d.ap(), X3t[:])
                nc.sync.dma_start(dbgx4_d.ap(), X4t[:])

            # ---- MLP layer 1: expert-pair windows over the sorted batch ----
            # Samples arrive expert-sorted; pair p's window covers experts
            # 2p,2p+1 on every core (block-diagonal weights pick the correct
            # expert rows; off-pair overlap columns produce ignored garbage).
            for p in range(4):
                s0, e0 = wins[p]
                Wp = e0 - s0
                if Wp <= 0:
                    continue
                ptm = psm.tile([128, Wp], dt.float32, tag="mlp")
                for t in range(25):
                    nc.tensor.matmul(
                        ptm[:],
                        w1sb[:, (p * 25 + t) * 128 : (p * 25 + t) * 128 + 128],
                        X4t[:, t, s0:e0],
                        start=(t == 0),
                        stop=(t == 24),
                    )
                relu_copy(Ha[:, p, 0:Wp], ptm[:], bmsb[:, p : p + 1])

            # ---- MLP layers 2-5: block-diagonal expert pairs ----
            src, dst = Ha, Hb
            for l in range(4):
                for p in range(4):
                    s0, e0 = wins[p]
                    Wp = e0 - s0
                    if Wp <= 0:
                        continue
                    idx = l * 4 + p
                    ptm = psm.tile([128, Wp], dt.float32, tag="mlp")
                    nc.tensor.matmul(
                        ptm[:],
                        w25sb[:, 128 * idx : 128 * idx + 128],
                        src[:, p, 0:Wp],
                        start=True,
                        stop=True,
                    )
                    relu_copy(dst[:, p, 0:Wp], ptm[:], bmsb[:, (l + 1) * 4 + p : (l + 1) * 4 + p + 1])
                src, dst = dst, src

            # ---- MLP layer 6 (no relu) + incremental output DMA ----
            for p in range(4):
                s0, e0 = wins[p]
                Wp = e0 - s0
                if Wp <= 0:
                    continue
                pt6 = psm.tile([12, Wp], dt.float32, tag="mlp")
                nc.tensor.matmul(
                    pt6[:],
                    w6sb[:, 12 * p : 12 * p + 12],
                    src[:, p, 0:Wp],
                    start=True,
                    stop=True,
                )
                nc.vector.tensor_scalar(
                    OUTt[:, p, 0:Wp], pt6[:], b6sb[:, p : p + 1], None, ALU.add
                )
                nc.sync.dma_start(out_d.ap()[:, p, 0:Wp], OUTt[:, p, 0:Wp])

    nc.compile()
    return nc


_prog_cache = {}
LAST_RESULTS = None


def _expert_layout(rm_state):
    """Balanced expert->core assignment + per-core expert-sorted sample order.

    Returns (perm, core_samples, ocum, wins):
      perm[2048]      sample permutation; perm[k*256:(k+1)*256] = core k's
                      samples, sorted by expert
      core_samples    [core][expert] -> global sample indices (in order)
      ocum[E+1,cores] per-core cumulative group offsets
      wins            4 (start,end) compile-time column windows; on every
                      core, experts 2p,2p+1 lie within wins[p]
    """
    n = np.bincount(rm_state, minlength=E).astype(np.int64)
    base = n // NCORES
    extra = np.zeros((E, NCORES), np.int64)
    ptr = 0
    for e in range(E):
        for _ in range(int(n[e] % NCORES)):
            extra[e, ptr % NCORES] += 1
            ptr += 1
    Mct = base[:, None] + extra  # [E, NCORES]; column sums == 256
    order = np.argsort(rm_state, kind="stable")
    core_samples = [[] for _ in range(NCORES)]
    off = 0
    for e in range(E):
        idxs = order[off : off + n[e]]
        off += int(n[e])
        s = 0
        for k in range(NCORES):
            core_samples[k].append(idxs[s : s + Mct[e, k]])
            s += int(Mct[e, k])
    perm = np.concatenate([np.concatenate(cs) for cs in core_samples])
    ocum = np.zeros((E + 1, NCORES), np.int64)
    ocum[1:] = np.cumsum(Mct, axis=0)
    Bcum = np.zeros(E + 1, np.int64)
    Bcum[1:] = np.cumsum(base)
    wins = []
    for p in range(4):
        s0 = int(Bcum[2 * p])
        e0 = int(min(Bcum[2 * p + 2] + 2 * p + 2, BC))
        wins.append((s0, max(s0, e0)))
    return perm, core_samples, ocum, wins


def kernel(**inputs):
    state = np.asarray(inputs["state"], dtype=np.float32)
    rm_state = np.asarray(inputs["rm_state"]).astype(np.int64)
    perm, core_samples, ocum, wins = _expert_layout(rm_state)
    state = state[perm]
    k1 = np.asarray(inputs["k1"], dtype=np.float32)
    c1 = np.asarray(inputs["c1"], dtype=np.float32)
    k2 = np.asarray(inputs["k2"], dtype=np.float32)
    c2 = np.asarray(inputs["c2"], dtype=np.float32)
    k3 = np.asarray(inputs["k3"], dtype=np.float32)
    c3 = np.asarray(inputs["c3"], dtype=np.float32)
    Ws = [np.asarray(inputs[f"W{i}"], dtype=np.float32) for i in range(1, 7)]
    Bs = [np.asarray(inputs[f"B{i}"], dtype=np.float32) for i in range(1, 7)]

    # ---- host prep ----
    # space-to-depth: [core, b, (i,dy), (j,dx), c] -> [core, (dy,dx,c), b, i, j]
    v = state.reshape(NCORES, BC, 21, 4, 21, 4, 3)
    s2d = np.ascontiguousarray(np.transpose(v, (0, 3, 5, 6, 1, 2, 4))).reshape(
        NCORES, 48, BC, 21, 21
    )
    # chunk c processes samples [32c, 32c+32): first 16 via x1a, next 16 via
    # x1b. Rows 48-95 duplicate rows 0-47 shifted one column in j (folds the
    # kx phase into the contraction dim -> K=96, 2 accumulation steps).
    vv = s2d.reshape(NCORES, 48, 8, 2, 16, 21, 21)
    halves = []
    bases = []
    for hh in range(2):
        base = np.ascontiguousarray(vv[:, :, :, hh]).reshape(NCORES, 48, 128, 21, 21)
        bases.append(base.astype(BF16))
        dup = np.zeros((NCORES, 96, 128, 21, 21), np.float32)
        dup[:, 0:48] = base
        dup[:, 48:96, :, :, 0:20] = base[:, :, :, :, 1:21]
        halves.append(dup.astype(BF16))
    x1a, x1b = halves
    x1an, x1bn = bases

    # conv weights: partition=(dy,dx,ci), free=(step, co)
    k1s = (
        k1.reshape(32, 3, 2, 4, 2, 4)
        .transpose(2, 4, 3, 5, 1, 0)
        .reshape(4, 48, 32)
    )  # steps (a, a2)
    k1r = np.zeros((96, 64), np.float32)
    for a in range(2):
        k1r[0:48, 32 * a : 32 * a + 32] = k1s[2 * a]
        k1r[48:96, 32 * a : 32 * a + 32] = k1s[2 * a + 1]
    k1r = k1r.astype(BF16)
    # base-layout conv1 weights: 4 (ky,kx) steps, K=48
    k1n = np.ascontiguousarray(k1s.transpose(1, 0, 2)).reshape(48, 128).astype(BF16)
    k2r = (
        k2.reshape(64, 32, 2, 2, 2, 2)
        .transpose(2, 4, 3, 5, 1, 0)
        .reshape(4, 128, 64)
        .transpose(1, 0, 2)
        .reshape(128, 256)
        .astype(BF16)
    )
    t3 = k3.transpose(2, 3, 1, 0)  # [ky, kx, ci, co]
    k3r = np.zeros((128, 384), np.float32)
    for kx in range(3):
        k3r[0:64, 64 * kx : 64 * kx + 64] = t3[0, kx]
        k3r[64:128, 64 * kx : 64 * kx + 64] = t3[1, kx]
        k3r[0:64, 192 + 64 * kx : 192 + 64 * kx + 64] = t3[2, kx]
    k3r = k3r.astype(BF16)

    # W1: [E, 3136, 64] -> s-pair tiles [partition=(spar,c), (pair, t, e2*64+m)]
    w1v = np.ascontiguousarray(
        Ws[0].reshape(4, 2, 64, 49, 64).transpose(0, 3, 2, 1, 4)
    ).reshape(4, 49, 64, 128)
    w1r = np.zeros((4, 25, 128, 128), np.float32)
    w1r[:, :, 0:64] = w1v[:, 0::2]
    w1r[:, :24, 64:128] = w1v[:, 1::2]
    # -> [128, 4*25*128] partition-major
    w1r = np.ascontiguousarray(w1r.transpose(2, 0, 1, 3)).reshape(128, 4 * 25 * 128).astype(BF16)

    # W2..W5 block-diagonal expert pairs: [128, 16*128] (col = (l*4+p)*128 + m')
    w25 = np.zeros((128, 16, 128), np.float32)
    for l in range(4):
        Wl = Ws[1 + l]
        for p in range(4):
            for e2 in range(2):
                w25[64 * e2 : 64 * e2 + 64, l * 4 + p, 64 * e2 : 64 * e2 + 64] = Wl[
                    2 * p + e2
                ]
    w25 = w25.reshape(128, 16 * 128).astype(BF16)

    w6 = np.zeros((128, 4, 12), np.float32)
    for p in range(4):
        for e2 in range(2):
            w6[64 * e2 : 64 * e2 + 64, p, 6 * e2 : 6 * e2 + 6] = Ws[5][2 * p + e2]
    w6 = w6.reshape(128, 48).astype(BF16)

    c1t = np.tile(c1, 4)[:, None].astype(np.float32)
    c2t = c2[:, None].astype(np.float32)
    c3t = c3[:, None].astype(np.float32)
    bmlp = np.zeros((128, 20), np.float32)
    for l in range(5):
        Bl = Bs[l]
        for p in range(4):
            bmlp[:, l * 4 + p] = np.concatenate([Bl[2 * p], Bl[2 * p + 1]])
    b6t = np.zeros((12, 4), np.float32)
    for p in range(4):
        b6t[:, p] = np.concatenate([Bs[5][2 * p], Bs[5][2 * p + 1]])

    # ---- build + run ----
    trace = bool(os.environ.get("NN_KERNEL_TRACE"))
    _install_axon_prof_shim()
    key = (tuple(wins), NODUP)
    if key not in _prog_cache:
        _prog_cache[key] = _build_program(wins, nodup=NODUP)
    nc = _prog_cache[key]

    shared = {
        "k1r": k1r,
        "k1n": k1n,
        "k2r": k2r,
        "k3r": k3r,
        "w1r": w1r,
        "w25r": w25,
        "w6r": w6,
        "c1t": c1t,
        "c2t": c2t,
        "c3t": c3t,
        "bmlp": bmlp,
        "b6t": b6t,
    }
    in_maps = [
        {
            "x1a": np.ascontiguousarray(x1a[c]),
            "x1b": np.ascontiguousarray(x1b[c]),
            "x1an": np.ascontiguousarray(x1an[c]),
            "x1bn": np.ascontiguousarray(x1bn[c]),
            **shared,
        }
        for c in range(NCORES)
    ]

    from concourse.bass_utils import run_bass_kernel_spmd

    res = run_bass_kernel_spmd(
        nc, in_maps, core_ids=list(range(NCORES)), trace=trace
    )
    if trace and res.exec_time_ns is not None:
        print(f"HW exec time: {res.exec_time_ns} ns")

    global LAST_RESULTS
    LAST_RESULTS = res.results

    # ---- host gather: pick each sample's own expert rows/cols ----
    WMAX = max(e0 - s0 for s0, e0 in wins)
    result = np.zeros((B, A), np.float32)
    for k in range(NCORES):
        out_k = np.asarray(res.results[k]["out"]).reshape(12, 4, WMAX)
        for e in range(E):
            idxs = core_samples[k][e]
            if len(idxs) == 0:
                continue
            p = e // 2
            s0, _ = wins[p]
            col0 = int(ocum[e, k]) - s0
            rows = slice((e % 2) * A, (e % 2) * A + A)
            result[idxs] = out_k[rows, p, col0 : col0 + len(idxs)].T
    return result

